# revision 1
# baseline (speedup 1.0000x reference)
"""Trainium2 Bass kernel for nn_FT_init: pixel_unshuffle -> FFT2 -> all-pairs
magnitude/phase recombination -> IFFT2 -> pixel_shuffle.

Strategy: pure data parallel, one sample per NeuronCore (8 cores).
All FFTs are 128x128 DFTs done as PE matmuls with the (symmetric) DFT
matrices as the *moving* operand and the data as the *stationary* operand,
which chains both sides of F X F^T with zero on-chip transposes.
Per (i,p) output block, two complex IFFTs recover 4 real phase-channels
(j = 4p+q, q=0..3) via Re/Im pairing.  fp32r (TF32-like) matmul dtype runs
at full PE rate with ~2e-4 rounding error.
"""
import sys

sys.path.insert(0, "/opt/trn_rl_repo")

import numpy as np
import concourse.bass as bass  # noqa: E402
import concourse.mybir as mybir  # noqa: E402
import concourse.tile as tile  # noqa: E402
import concourse.bacc as bacc  # noqa: E402
from concourse.bass_utils import run_bass_kernel_spmd  # noqa: E402

F32 = mybir.dt.float32
F32R = mybir.dt.float32r
BF16 = mybir.dt.bfloat16
N = 128  # DFT size
R = 4  # msfa / pixel shuffle factor
C = 16  # channels = R*R
MULT = mybir.AluOpType.mult
ADD = mybir.AluOpType.add
SUB = mybir.AluOpType.subtract
SQUARE = mybir.ActivationFunctionType.Square


def _dft_consts():
    import ml_dtypes
    k = np.arange(N)
    ang = 2.0 * np.pi / N * np.outer(k, k)
    Wr = np.cos(ang).astype(np.float32)
    Wi = (-np.sin(ang)).astype(np.float32)
    Gr = (np.cos(ang) / N).astype(np.float32)
    Gi = (np.sin(ang) / N).astype(np.float32)
    # column-interleaved inverse consts: IGG[:, 2n+q] = (Gr|Gi)[:, n],
    # IGG2[:, 2n+q] = (-Gi|Gr)[:, n] -> stage-2 matmul output comes out
    # pre-pixel-shuffled in PSUM.
    IGG = np.empty((N, 256), np.float32)
    IGG[:, 0::2] = Gr
    IGG[:, 1::2] = Gi
    IGG2 = np.empty((N, 256), np.float32)
    IGG2[:, 0::2] = -Gi
    IGG2[:, 1::2] = Gr
    cin = np.hstack([Wr, Wi, -Wi, Wr, IGG, IGG2]).astype(np.float32)
    cinb = np.hstack([Gr, Gi, -Gi, Gr]).astype(ml_dtypes.bfloat16)
    cinf = np.hstack([Gr, Gi, -Gi, Gr]).astype(np.float32)
    return cin, cinb, cinf


REPEAT = 1  # >1 only for timing experiments
TIMING_MODE = False  # True: big output stays on-device (for wall-time diffs)
CMUL_GP = 24  # how many of the 32 C-mul ops run on GPSIMD (rest on DVE)


def _build():
    nc = bacc.Bacc("TRN2", target_bir_lowering=False, debug=False, num_devices=8)
    xin = nc.dram_tensor("xin", [128, 2048], F32R, kind="ExternalInput")
    cin = nc.dram_tensor("cin", [128, 1024], F32R, kind="ExternalInput")
    cinb = nc.dram_tensor("cinb", [128, 512], BF16, kind="ExternalInput")
    cinf = nc.dram_tensor("cinf", [128, 512], F32R, kind="ExternalInput")
    if TIMING_MODE:
        outd = nc.dram_tensor("outd", [C, 128, 2048], F32, kind="Internal")
        tiny = nc.dram_tensor("tiny", [1, 128], F32, kind="ExternalOutput")
    else:
        outd = nc.dram_tensor("outd", [C, 128, 2048], F32, kind="ExternalOutput")

    with tile.TileContext(nc) as tc:
        with (
            tc.tile_pool(name="persist", bufs=1) as pp,
            tc.tile_pool(name="ew", bufs=2) as ew,
            tc.tile_pool(name="sx", bufs=3) as sxp,
            tc.tile_pool(name="cpool", bufs=8) as cpool,
            tc.tile_pool(name="spool", bufs=8) as spool,
            tc.tile_pool(name="oimg", bufs=6) as oimg,
            tc.tile_pool(name="ps1", bufs=4, space="PSUM") as ps1,
            tc.tile_pool(name="ps2", bufs=4, space="PSUM") as ps2,
        ):
            consts = pp.tile([128, 1024], F32R)
            nc.sync.dma_start(consts[:], cin[:, :])
            constsb = pp.tile([128, 512], BF16)
            nc.sync.dma_start(constsb[:], cinb[:, :])
            constsf = pp.tile([128, 512], F32R)
            nc.sync.dma_start(constsf[:], cinf[:, :])
            WW = consts[:, 0:256]
            WW2 = consts[:, 256:512]
            IGG = consts[:, 512:768]
            IGG2 = consts[:, 768:1024]
            GGb = constsb[:, 0:256].bitcast(BF16)  # unused in f32r variant
            GG2b = constsb[:, 256:512]
            GG = constsf[:, 0:256]
            GG2 = constsf[:, 256:512]

            xrows = pp.tile([128, 2048], F32R)
            nc.sync.dma_start(xrows[:], xin[:, :])

            for _rep in range(REPEAT):
                # forward FFT2, 4 channels per group: Z = W X W (W symmetric).
                # Channel c=(p,q) is read straight out of xrows with a
                # stride-4 stationary AP (pixel-unshuffle fused into LDW).
                z_all = pp.tile([128, 4096], F32)
                mag = pp.tile([128, 2048], F32)
                v_all = pp.tile([128, 2048], F32)
                def fwd_group(g):
                    # 2 channels per group -> [128,512] psum tiles (1 bank)
                    ps_a = ps1.tile([128, 512], F32, tag="s1")
                    for cc in range(2):
                        c = g * 2 + cc
                        p, q = divmod(c, R)
                        xs = bass.AP(
                            xrows[:].tensor,
                            xrows[:].offset + p * 512 + q,
                            [xrows[:].ap[0], [4, 128]],
                        )
                        nc.tensor.matmul(ps_a[:, cc * 256:(cc + 1) * 256], xs, WW,
                                         start=True, stop=True)
                    sx = sxp.tile([128, 512], F32R)
                    nc.scalar.copy(sx[:], ps_a[:])
                    ps_b = ps2.tile([128, 512], F32, tag="s2")
                    for cc in range(2):
                        o = cc * 256
                        nc.tensor.matmul(ps_b[:, o:o + 256], sx[:, o:o + 128], WW,
                                         start=True, stop=False)
                        nc.tensor.matmul(ps_b[:, o:o + 256], sx[:, o + 128:o + 256],
                                         WW2, start=False, stop=True)
                    nc.vector.tensor_copy(z_all[:, g * 512:(g + 1) * 512], ps_b[:])

                def mag_chunk(p):
                    zc = z_all[:, p * 1024:(p + 1) * 1024]
                    zvv = zc.rearrange("z (c h n) -> z h c n", h=2, n=128)
                    zr, zi = zvv[:, 0], zvv[:, 1]  # [128, 4, 128] views
                    t1 = ew.tile([128, 512], F32, tag="t1")
                    t2 = ew.tile([128, 512], F32, tag="t2")
                    t1v = t1[:].rearrange("z (c n) -> z c n", n=128)
                    t2v = t2[:].rearrange("z (c n) -> z c n", n=128)
                    nc.vector.tensor_tensor(t1v, zr, zr, MULT)
                    nc.scalar.activation(t2v, zi, SQUARE)
                    sq = ew.tile([128, 512], F32, tag="sq")
                    nc.gpsimd.tensor_add(sq[:], t1[:], t2[:])
                    mgf = mag[:, p * 512:(p + 1) * 512]
                    nc.scalar.sqrt(mgf, sq[:])
                    rmag = ew.tile([128, 512], F32, tag="rmag")
                    scr = ew.tile([128, 512], F32, tag="scr")
                    nc.vector.reciprocal_approx_accurate(rmag[:], mgf, scr[:])
                    ur = ew.tile([128, 512], F32, tag="ur")
                    ui = ew.tile([128, 512], F32, tag="ui")
                    urv = ur[:].rearrange("z (c n) -> z c n", n=128)
                    uiv = ui[:].rearrange("z (c n) -> z c n", n=128)
                    rmv = rmag[:].rearrange("z (c n) -> z c n", n=128)
                    nc.vector.tensor_tensor(urv, zr, rmv, MULT)
                    nc.gpsimd.tensor_tensor(uiv, zi, rmv, MULT)
                    # pairs within this p-group: t = 2p+h, j0 = 4p+2h, j1 = j0+1
                    # layout: v_all[:, p*512 + h*256 + g*128 + n], g = re/im
                    vv = v_all[:, p * 512:(p + 1) * 512].rearrange(
                        "z (h g n) -> z g h n", g=2, n=128)
                    ur0 = bass.AP(ur[:].tensor, ur[:].offset, [ur[:].ap[0], [256, 2], [1, 128]])
                    ui0 = bass.AP(ui[:].tensor, ui[:].offset, [ui[:].ap[0], [256, 2], [1, 128]])
                    ur1 = bass.AP(ur[:].tensor, ur[:].offset + 128, [ur[:].ap[0], [256, 2], [1, 128]])
                    ui1 = bass.AP(ui[:].tensor, ui[:].offset + 128, [ui[:].ap[0], [256, 2], [1, 128]])
                    nc.vector.tensor_tensor(vv[:, 0], ur0, ui1, SUB)
                    nc.vector.tensor_tensor(vv[:, 1], ui0, ur1, ADD)

                # inverse, ph-major: see inv_iter
                def inv_iter(ph, i, it):
                    c_t = cpool.tile([128, 1024], F32R)
                    cv = c_t[:].rearrange("z (a n) -> z a n", a=8)
                    vv = v_all[:, ph * 1024:(ph + 1) * 1024].rearrange(
                        "z (a n) -> z a n", a=8)
                    mb = mag[:, i * 128:(i + 1) * 128][:, None, :].broadcast_to(
                        [128, 8, 128])
                    if it % 8 in (2, 5, 7):
                        nc.vector.tensor_tensor(cv, vv, mb, MULT)
                    else:
                        nc.gpsimd.tensor_tensor(cv, vv, mb, MULT)

                    outh = oimg.tile([128, 1024], F32)
                    for half in range(2):
                        o = half * 512
                        s1 = ps1.tile([128, 512], F32, tag="s1")
                        nc.tensor.matmul(s1[:, 0:256], c_t[:, o:o + 128],
                                         GG, start=True, stop=False)
                        nc.tensor.matmul(s1[:, 0:256], c_t[:, o + 128:o + 256],
                                         GG2, start=False, stop=True)
                        nc.tensor.matmul(s1[:, 256:512], c_t[:, o + 256:o + 384],
                                         GG, start=True, stop=False)
                        nc.tensor.matmul(s1[:, 256:512], c_t[:, o + 384:o + 512],
                                         GG2, start=False, stop=True)

                        s_t = spool.tile([128, 512], F32R)
                        if (2 * it + half) % 8 < 3:
                            nc.vector.tensor_copy(s_t[:], s1[:])
                        else:
                            nc.scalar.copy(s_t[:], s1[:])

                        s2t = ps2.tile([128, 512], F32, tag="s2")
                        for b in range(2):
                            # out cols 4n + 2b + q, q in {0,1}
                            oap = bass.AP(
                                s2t[:].tensor, s2t[:].offset + 2 * b,
                                [s2t[:].ap[0], [4, 128], [1, 2]])
                            sb = b * 256
                            nc.tensor.matmul(oap, s_t[:, sb:sb + 128],
                                             IGG, start=True, stop=False)
                            nc.tensor.matmul(oap, s_t[:, sb + 128:sb + 256],
                                             IGG2, start=False, stop=True)

                        # pre-interleaved in PSUM: contiguous eviction
                        if (2 * it + half) % 2 == 0:
                            nc.scalar.copy(outh[:, o:o + 512], s2t[:])
                        else:
                            nc.vector.tensor_copy(outh[:, o:o + 512], s2t[:])

                    nc.sync.dma_start(outd[i, :, ph * 1024:(ph + 1) * 1024], outh[:])

                fwd_group(0)
                fwd_group(1)
                mag_chunk(0)
                fwd_group(2)
                fwd_group(3)
                mag_chunk(1)
                fwd_group(4)
                fwd_group(5)
                mag_chunk(2)
                fwd_group(6)
                fwd_group(7)
                mag_chunk(3)
                for it, (ph, i) in enumerate(
                        [(0, i) for i in range(C)] + [(1, i) for i in range(C)]):
                    inv_iter(ph, i, it)

            if TIMING_MODE:
                nc.sync.dma_start(tiny[:, :], consts[0:1, 0:128].bitcast(F32))

    nc.compile()
    return nc


_NC = None


def _get_nc():
    global _NC
    if _NC is None:
        _NC = _build()
    return _NC


def kernel(x: np.ndarray) -> np.ndarray:
    x = np.asarray(x, dtype=np.float32)
    assert x.shape == (8, 1, 512, 512), x.shape
    nc = _get_nc()
    cin, cinb, cinf = _dft_consts()
    in_maps = [
        {"xin": np.ascontiguousarray(x[b, 0].reshape(128, 2048)), "cin": cin,
         "cinb": cinb, "cinf": cinf}
        for b in range(8)
    ]
    res = run_bass_kernel_spmd(nc, in_maps, core_ids=list(range(8)))
    out = np.stack([r["outd"].reshape(C, 512, 512) for r in res.results])
    return out.astype(np.float32)


if __name__ == "__main__":
    rng = np.random.RandomState(0)
    x = rng.randn(8, 1, 512, 512).astype(np.float32)
    y = kernel(x)
    print(y.shape, y.dtype)

